# revision 34
# baseline (speedup 1.0000x reference)
"""Trainium2 Bass kernel for nn_Attention_29738353557815.

8-way tensor-parallel over heads:
  - core c owns q-heads {2c, 2c+1} and kv-head c//2 (k/v proj duplicated per core pair)
  - projections run weights-stationary off a host-pretransposed hidden^T, producing
    q/k in [head_dim, T] layout; v is produced transposed then PE-transposed back
  - rms-norm folded into ln/exp on ACT; rope tables (cos/sin * norm_w * sqrt(scale))
    are host-precomputed in [hd, T] layout; rotate-half via half-tile tensor_tensor
    ops against a half-swapped sin table
  - attention computed in S^T layout ([key, query] tiles): causal mask via
    gpsimd.affine_select, segment mask via scalar_tensor_tensor against iota;
    invalid (s,t) tiles are skipped entirely at build time (segment sparsity)
  - softmax denominator via ones-matmul column sums; normalization and sigmoid
    gating fused into one multiply before the o-projection
  - AllToAll (2 MiB/rank) redistributes gated attention so each core computes
    output rows [256c, 256c+256) with the full wo; host concatenates

All DMAs are arranged for >=4 KiB contiguous per-partition runs (weights are
host-prepacked into [128, ...] partition-major layouts) — smaller runs hit the
~200ns/descriptor DMA floor and halve effective bandwidth.
"""
import sys

if "/opt/trn_rl_repo" not in sys.path:
    sys.path.insert(0, "/opt/trn_rl_repo")

import numpy as np

import concourse.bass as bass
from concourse import bacc
import concourse.mybir as mybir
import concourse.tile as tile
from concourse.bass_utils import run_bass_kernel_spmd
from concourse.masks import make_identity

F32 = mybir.dt.float32
F32R = mybir.dt.float32r
BF16 = mybir.dt.float16  # fp16: same DMA savings as bf16, 4x finer mantissa
BF16_OPROJ = True  # o-projection pipeline (wo, A2A payload) in fp16
AF = mybir.ActivationFunctionType
OP = mybir.AluOpType

B, T, D = 1, 2048, 2048
NH, NKV, HD = 16, 4, 128
EPS = 1e-6
SCALE = HD ** -0.5
NCORES = 8
P = 128
NJ = T // 512      # 4 t-chunks of 512
NT = T // P        # 16 s-tiles of 128
DT = D // P        # 16 contraction tiles
TSL = T // NCORES  # 256 output rows per core

_program_cache: dict = {}


def _tile_flags(seg_end: np.ndarray):
    """Per (s-tile i, t-chunk j): (skip, needs_causal, needs_seg)."""
    flags = []
    for i in range(NT):
        smin, smax = P * i, P * i + P - 1
        se_lo = int(seg_end[smin])
        se_hi = int(seg_end[smax])
        row = []
        for j in range(NJ):
            t0, t1 = 512 * j, 512 * j + 511
            skip = (t1 < smin) or (t0 >= se_hi)
            causal = (not skip) and (t0 < smax)
            segm = (not skip) and (t1 >= se_lo)
            row.append((skip, causal, segm))
        flags.append(row)
    return tuple(tuple(r) for r in flags)


def _build_program(key, use_collective=True):
    flags, unit_w = key
    nc = bacc.Bacc("TRN2", target_bir_lowering=False, debug=False,
                   num_devices=NCORES)

    hT_d = nc.dram_tensor("hT", [D, T], F32R, kind="ExternalInput")
    # host-prepacked partition-major weights (see _host_prep)
    wqg_d = nc.dram_tensor("wqg", [P, DT, 512], F32R, kind="ExternalInput")
    wkv_d = nc.dram_tensor("wkv", [P, DT, 256], F32R, kind="ExternalInput")
    if BF16_OPROJ:
        wo_d = nc.dram_tensor("wo", [P, NT, 2048], BF16, kind="ExternalInput")
    else:
        wo_d = nc.dram_tensor("wo", [P, NT, 2, 1024], F32R, kind="ExternalInput")
    ODT = BF16 if BF16_OPROJ else F32R
    tblq_d = nc.dram_tensor("tblq", [2, P, T], F32, kind="ExternalInput")
    if not unit_w:
        wqk_d = nc.dram_tensor("wqk", [P, 2], F32, kind="ExternalInput")
    iota_d = nc.dram_tensor("iota", [P, 512], F32, kind="ExternalInput")
    segrel_d = nc.dram_tensor("segrel", [P, NT, NJ], F32, kind="ExternalInput")
    out_d = nc.dram_tensor("out", [TSL, D], F32, kind="ExternalOutput")

    hT_re = hT_d.rearrange("(dt p) t -> p dt t", p=P)

    hw_b, tmp_b, ptp_b, tmp2_b, atall_b = 16, 5, 5, 2, 8
    with tile.TileContext(nc) as tc:
        with (
            tc.tile_pool(name="consts", bufs=1) as consts,
            tc.tile_pool(name="perm", bufs=1) as perm,
            tc.tile_pool(name="hw", bufs=hw_b) as hw,
            tc.tile_pool(name="tmp", bufs=tmp_b) as tmp,
            tc.tile_pool(name="ptp", bufs=ptp_b) as ptp,
            tc.tile_pool(name="ps", bufs=1, space="PSUM") as psp,
            tc.tile_pool(name="dram", bufs=1, space="DRAM") as dram,
        ):
            # ---- constants; DMA emission of the big ones is interleaved with
            # the first hT tiles inside phase B so the first matmul starts early
            wqg_sb = [consts.tile([P, 4, 512], F32R, tag="wqg", bufs=4,
                                  name=f"wqg{g}") for g in range(4)]
            wkv_sb = [consts.tile([P, 8, 256], F32R, tag="wkv", bufs=2,
                                  name=f"wkv{g}") for g in range(2)]

            def wq_ap(dt, col0):
                return wqg_sb[dt // 4][:, dt % 4, col0:col0 + 128]

            def wkv_ap(dt, col0):
                return wkv_sb[dt // 8][:, dt % 8, col0:col0 + 128]

            tb = {}
            tb_srcs = []
            for nm, idx in (("cq", 0), ("sq", 1)):
                t_ = consts.tile([P, T], F32, tag=f"tb_{nm}", name=f"tb_{nm}")
                tb_srcs.append((t_, tblq_d, idx))
                tb[nm] = t_
            tb["ck"], tb["sk"] = tb["cq"], tb["sq"]
            if not unit_w:
                wqk_sb = consts.tile([P, 2], F32)
                nc.sync.dma_start(wqk_sb[:], wqk_d[:])
            iota_sb = consts.tile([P, 512], F32)
            segrel_sb = consts.tile([P, NT, NJ], F32)
            ones_f32 = consts.tile([P, P], F32)
            nc.vector.memset(ones_f32[:], 1.0)
            ones_sb = consts.tile([P, P], F32R)
            nc.vector.tensor_copy(ones_sb[:], ones_f32[:])
            ident_sb = consts.tile([P, P], F32)
            make_identity(nc, ident_sb[:])
            eps_sb = consts.tile([P, 1], F32)
            nc.vector.memset(eps_sb[:], EPS)

            # ---- persistent activations ----
            qTr = [perm.tile([P, T], F32R, tag=f"qTr{h}", name=f"qTr{h}")
                   for h in range(2)]
            kTr = perm.tile([P, T], F32R, tag="kTr")
            gT = [perm.tile([P, T], F32, tag=f"gT{h}", name=f"gT{h}")
                  for h in range(2)]
            v_sb = perm.tile([P, NT, P], F32R, tag="v_sb")

            # split A2A by head: h0's collective runs while h1 attention computes
            a2a_in = [dram.tile([NCORES * P, TSL], ODT, name=f"a2a_in{h}")
                      for h in range(2)]
            a2a_in8 = [a.rearrange("(s r) t -> s r t", r=P) for a in a2a_in]
            a2a_out = [dram.tile([NCORES * P, TSL], ODT, name=f"a2a_out{h}")
                       for h in range(2)]

            def emit_attention(h, j):
                tsl = slice(512 * j, 512 * j + 512)
                valid = [i for i in range(NT) if not flags[i][j][0]]
                last = len(valid) - 1
                ot_ps = psp.tile([P, 512], F32, tag="acc", bufs=4,
                                 name=f"ot_{h}_{j}")
                rs_ps = psp.tile([P, 512], F32, tag="acc", bufs=4,
                                 name=f"rs_{h}_{j}")
                for idx, i in enumerate(valid):
                    _, needs_c, needs_s = flags[i][j]
                    st_ps = psp.tile([P, 512], F32, tag="mm", bufs=3,
                                     name=f"st_{h}_{j}_{i}")
                    nc.tensor.matmul(st_ps[:], kTr[:, P * i:P * i + P],
                                     qTr[h][:, tsl], start=True, stop=True)
                    pt = ptp.tile([P, 512], F32R, tag="pt", name=f"pt_{h}_{j}_{i}")
                    nc.scalar.activation(pt[:], st_ps[:], AF.Exp)
                    if needs_c:
                        nc.gpsimd.affine_select(
                            out=pt[:], in_=pt[:], pattern=[[1, 512]],
                            compare_op=OP.is_ge, fill=0.0,
                            base=512 * j - P * i, channel_multiplier=-1)
                    if needs_s:
                        nc.vector.scalar_tensor_tensor(
                            out=pt[:], in0=iota_sb[:],
                            scalar=segrel_sb[:, i, j:j + 1], in1=pt[:],
                            op0=OP.is_lt, op1=OP.mult)
                    nc.tensor.matmul(ot_ps[:], v_sb[:, i, :], pt[:],
                                     start=(idx == 0), stop=(idx == last))
                    nc.tensor.matmul(rs_ps[:], ones_sb[:], pt[:],
                                     start=(idx == 0), stop=(idx == last))

                # sig(g)/rowsum = exp(-(ln(1+e^-g) + ln(rowsum)));
                # gT already holds ln(1+e^-g) from phase B
                sg = tmp.tile([P, 512], F32, tag="tmp", name=f"sg_{h}_{j}")
                nc.scalar.activation(sg[:], rs_ps[:], AF.Ln)
                nc.vector.tensor_tensor(sg[:], sg[:], gT[h][:, tsl], OP.add)
                nc.scalar.activation(sg[:], sg[:], AF.Exp, scale=-1.0)
                ot_sb = tmp.tile([P, 512], F32, tag="tmp", name=f"otsb_{h}_{j}")
                nc.vector.tensor_copy(ot_sb[:], ot_ps[:])
                atg = tmp.tile([P, 512], ODT, tag="tmp2", bufs=tmp2_b,
                               name=f"atg_{h}_{j}")
                nc.vector.tensor_tensor(atg[:], ot_sb[:], sg[:], OP.mult)
                # stage into a2a_in[h]: chunk j covers shards 2j and 2j+1
                for half in range(2):
                    nc.sync.dma_start(
                        a2a_in8[h][2 * j + half, :, :],
                        atg[:, 256 * half:256 * half + 256])


            # ================= phase B: projections =================
            # t-halves of 1024 so hT tiles have 4 KiB runs at tolerable SBUF cost
            for half in range(2):
                hTt = []
                for dt in range(DT):
                    if half == 0 and dt % 4 == 0:
                        g = dt // 4
                        nc.sync.dma_start(wqg_sb[g][:],
                                          wqg_d[:, 4 * g:4 * g + 4, :])
                    t_ = hw.tile([P, 1024], F32R, tag="hw", name=f"hT_{half}_{dt}")
                    nc.sync.dma_start(
                        t_[:], hT_re[:, dt, 1024 * half:1024 * half + 1024])
                    hTt.append(t_)
                if half == 0:
                    for g in range(2):
                        nc.sync.dma_start(wkv_sb[g][:],
                                          wkv_d[:, 8 * g:8 * g + 8, :])
                    for t_, dsrc, idx in tb_srcs:
                        nc.sync.dma_start(t_[:], dsrc[idx])
                    nc.sync.dma_start(iota_sb[:], iota_d[:])
                    nc.sync.dma_start(segrel_sb[:], segrel_d[:])
                for jj in range(2):
                    j = 2 * half + jj
                    tsl = slice(512 * j, 512 * j + 512)
                    hsl = slice(512 * jj, 512 * jj + 512)

                    # order: q0 q1 g0 g1 k v (k/v last -> slack for wkv DMA)
                    for c in (0, 1, 4, 5, 2, 3):
                        if c < 2:
                            w_ap = lambda dt, c=c: wq_ap(dt, 128 * c)
                        elif c == 2:
                            w_ap = lambda dt: wkv_ap(dt, 0)
                        elif c == 3:
                            w_ap = lambda dt: wkv_ap(dt, 128)
                        else:
                            w_ap = lambda dt, c=c: wq_ap(dt, 256 + 128 * (c - 4))

                        ptag, pbufs = (("mm", 3) if c in (0, 1, 4, 5) else ("acc", 4))
                        mm_ps = psp.tile([P, 512], F32, tag=ptag, bufs=pbufs,
                                         name=f"proj_{j}_{c}")
                        for dt in range(DT):
                            nc.tensor.matmul(mm_ps[:], w_ap(dt), hTt[dt][:, hsl],
                                             start=(dt == 0), stop=(dt == DT - 1))

                        if c in (0, 1, 2):  # q0/q1/k: rms-norm + rope
                            dest = qTr[c][:, tsl] if c < 2 else kTr[:, tsl]
                            cosw = tb["cq"] if c < 2 else tb["ck"]
                            sinw = tb["sq"] if c < 2 else tb["sk"]
                            qpre = tmp.tile([P, 512], F32, tag="tmp")
                            nc.vector.tensor_copy(qpre[:], mm_ps[:])
                            q2 = tmp.tile([P, 512], F32R, tag="tmp2", bufs=tmp2_b)
                            nc.scalar.activation(q2[:], mm_ps[:], AF.Square)
                            if not unit_w:
                                # norm weight applied after the rms statistic,
                                # before rope (rope commutes with rsqrt only)
                                qw = tmp.tile([P, 512], F32, tag="tmp")
                                nc.vector.tensor_scalar_mul(
                                    qw[:], qpre[:],
                                    wqk_sb[:, (0 if c < 2 else 1):
                                           (1 if c < 2 else 2)])
                                qpre = qw
                            ssq_ps = psp.tile([P, 512], F32, tag="aux", bufs=1)
                            nc.tensor.matmul(ssq_ps[:], ones_sb[:], q2[:],
                                             start=True, stop=True)
                            rsv = tmp.tile([P, 512], F32, tag="tmp")
                            nc.scalar.activation(rsv[:], ssq_ps[:], AF.Ln,
                                                 scale=1.0 / HD, bias=eps_sb[:, 0:1])
                            nc.scalar.activation(rsv[:], rsv[:], AF.Exp, scale=-0.5)
                            tcos = tmp.tile([P, 512], F32, tag="tmp")
                            nc.vector.tensor_tensor(tcos[:], qpre[:], cosw[:, tsl],
                                                    OP.mult)
                            t2 = tmp.tile([P, 512], F32, tag="tmp")
                            # sin table halves are pre-swapped host-side so both
                            # inputs share a base partition; only out is shifted
                            nc.vector.tensor_tensor(t2[0:64, :], qpre[64:128, :],
                                                    sinw[64:128, tsl], OP.mult)
                            nc.vector.tensor_tensor(t2[64:128, :], qpre[0:64, :],
                                                    sinw[0:64, tsl], OP.mult)
                            nc.vector.tensor_tensor(t2[:], tcos[:], t2[:], OP.add)
                            nc.vector.tensor_tensor(dest, t2[:], rsv[:], OP.mult)
                        elif c in (4, 5):  # gate: store ln(1+exp(-g))
                            eg = tmp.tile([P, 512], F32, tag="tmp")
                            nc.scalar.activation(eg[:], mm_ps[:], AF.Exp,
                                                 scale=-1.0)
                            nc.scalar.activation(gT[c - 4][:, tsl], eg[:],
                                                 AF.Ln, bias=1.0)
                        else:  # v: transpose [hd, t] -> [t, hd] tiles
                            vtmp = tmp.tile([P, 512], F32, tag="tmp")
                            nc.vector.tensor_copy(vtmp[:], mm_ps[:])
                            for kk in range(4):
                                tt = 4 * j + kk
                                trp = psp.tile([P, P], F32, tag="aux", bufs=1)
                                nc.tensor.transpose(
                                    trp[:], vtmp[:, 128 * kk:128 * kk + 128],
                                    ident_sb[:])
                                nc.vector.tensor_copy(v_sb[:, tt, :], trp[:])

            for j in range(NJ):
                emit_attention(0, j)
            if use_collective:
                nc.gpsimd.collective_compute(
                    "AllToAll", OP.bypass,
                    replica_groups=[list(range(NCORES))],
                    ins=[a2a_in[0][:].opt()], outs=[a2a_out[0][:].opt()])
            else:
                nc.sync.dma_start(a2a_out[0][:], a2a_in[0][:])

            # h=1 attention (h=0 was fused into the projection loop); its
            # collective overlaps with nothing ahead of it, while h=0's
            # collective ran during these blocks
            for j in range(NJ):
                emit_attention(1, j)
            if use_collective:
                nc.gpsimd.collective_compute(
                    "AllToAll", OP.bypass,
                    replica_groups=[list(range(NCORES))],
                    ins=[a2a_in[1][:].opt()], outs=[a2a_out[1][:].opt()])
            else:
                nc.sync.dma_start(a2a_out[1][:], a2a_in[1][:])

            # ================= phase D: o-proj =================

            # o-proj, ht-major: all 8 PSUM banks accumulate [m 0/1] x [Dc 0..3];
            # ATall and wo tiles stream (wo shares the "hw" slots freed by hT)
            ops_tags = ["mm", "mm", "mm", "aux", "acc", "acc", "acc", "acc"]
            ops_bufs = {"mm": 3, "aux": 1, "acc": 4}
            ops = []
            for m in range(2):
                for Dc in range(NJ):
                    tg = ops_tags[m * NJ + Dc]
                    ops.append(psp.tile([P, 512], F32, tag=tg,
                                        bufs=ops_bufs[tg], name=f"ops{m}_{Dc}"))
            # ht-step order: all h0 blocks then all h1 blocks (matches the
            # two collectives' completion order; wo is host-packed to match)
            ATall = []
            for hs in range(NT):
                h, i = hs // 8, hs % 8
                at_t = perm.tile([P, TSL], ODT, tag="ATall", bufs=atall_b,
                                 name=f"ATall{hs}")
                nc.sync.dma_start(at_t[:], a2a_out[h][P * i:P * i + P, :])
                ATall.append(at_t)
            for ht in range(NT):
                at_t = ATall[ht]
                if BF16_OPROJ:
                    w_full = hw.tile([P, 2048], BF16, tag="hw", name=f"wo_{ht}")
                    nc.sync.dma_start(w_full[:], wo_d[:, ht, :])
                    wslices = [w_full[:, 512 * Dc:512 * Dc + 512]
                               for Dc in range(NJ)]
                else:
                    wslices = []
                    for Dh in range(2):
                        w_ = hw.tile([P, 1024], F32R, tag="hw",
                                     name=f"wo_{ht}_{Dh}")
                        nc.sync.dma_start(w_[:], wo_d[:, ht, Dh, :])
                        wslices += [w_[:, 0:512], w_[:, 512:1024]]
                for Dc in range(NJ):
                    for m in range(2):
                        nc.tensor.matmul(
                            ops[m * NJ + Dc][:],
                            at_t[:, 128 * m:128 * m + 128], wslices[Dc],
                            start=(ht == 0), stop=(ht == NT - 1))
            for m in range(2):
                for Dc in range(NJ):
                    dsl = slice(512 * Dc, 512 * Dc + 512)
                    o_sb = tmp.tile([P, 512], F32, tag="tmp")
                    nc.vector.tensor_copy(o_sb[:], ops[m * NJ + Dc][:])
                    nc.sync.dma_start(out_d[128 * m:128 * m + 128, dsl], o_sb[:])

    nc.compile()
    _dedupe_act_table_loads(nc)
    return nc


def _dedupe_act_table_loads(nc):
    """Bacc assigns Exp->exp_and_others and Ln->natural_log, inserting a
    ~2.7us table load at every Exp<->Ln alternation (57 of them here). All
    activation funcs this kernel uses (Exp, Ln, Square) live in the
    natural_log_exp_and_others set, so keep one load of that set and drop
    the rest."""
    from concourse.hw_specs import get_activation_tables
    tabs = list(get_activation_tables(nc.m.arch).items())
    nl_exp = next(i for i, (nm, funcs) in enumerate(tabs)
                  if nm == "natural_log_exp_and_others")
    used = {ins.func for bb in nc.main_func.blocks for ins in bb.instructions
            if isinstance(ins, mybir.InstActivation)}
    assert used <= tabs[nl_exp][1], f"funcs {used} not all in natural_log_exp"
    first = True
    for bb in nc.main_func.blocks:
        keep = []
        for ins in bb.instructions:
            if isinstance(ins, mybir.InstLoadActFuncSet):
                assert ins.sync_info is None or (
                    not ins.sync_info.on_wait and not ins.sync_info.on_update)
                if first:
                    ins.act_func_set_id = nl_exp
                    keep.append(ins)
                    first = False
                continue
            keep.append(ins)
        bb.instructions[:] = keep


def _host_prep(hidden_BTD, cos_BTK, sin_BTK, segment_ids_BT, position_ids_BT,
               wq, wk, wv, wo, q_norm_w, k_norm_w):
    hidden = np.ascontiguousarray(np.asarray(hidden_BTD, dtype=np.float32)[0])
    cos = np.asarray(cos_BTK, dtype=np.float32)[0]
    sin = np.asarray(sin_BTK, dtype=np.float32)[0]
    seg = np.asarray(segment_ids_BT)[0]
    pos = np.asarray(position_ids_BT)[0]
    wq = np.asarray(wq, dtype=np.float32)
    wk = np.asarray(wk, dtype=np.float32)
    wv = np.asarray(wv, dtype=np.float32)
    wo = np.asarray(wo, dtype=np.float32)
    q_norm_w = np.asarray(q_norm_w, dtype=np.float32)
    k_norm_w = np.asarray(k_norm_w, dtype=np.float32)

    assert np.array_equal(pos, np.arange(T, dtype=pos.dtype)), \
        "kernel assumes position_ids == arange"
    assert np.all(np.diff(seg) >= 0), "kernel assumes sorted segment ids"

    hT = np.ascontiguousarray(hidden.T)
    sqrtS = np.float32(np.sqrt(SCALE))
    signv = np.where(np.arange(HD) < HD // 2, -1.0, 1.0).astype(np.float32)
    shuf = (np.arange(HD) + HD // 2) % HD

    cosw = (cos.T * sqrtS).astype(np.float32)
    sinw = (sin.T * signv[:, None] * sqrtS).astype(np.float32)
    sinswap = sinw[shuf]  # halves swapped: see rotate-half ops in _build_program
    tblq = np.ascontiguousarray(np.stack([cosw, sinswap]))
    unit_w = bool(np.all(q_norm_w == 1.0) and np.all(k_norm_w == 1.0))
    wqk = np.ascontiguousarray(np.stack([q_norm_w, k_norm_w], axis=1))

    # prepack wo into partition-major layout; block order matches the
    # o-proj ht-step order (all h0 head-blocks, then all h1)
    perm = [2 * i + h for h in range(2) for i in range(NCORES)]
    if BF16_OPROJ:
        wo_p = wo.reshape(NT, P, 2048)[perm].transpose(1, 0, 2)
        wo_p = np.ascontiguousarray(wo_p.astype(np.float16))
    else:
        wo_p = np.ascontiguousarray(
            wo.reshape(NT, P, 2, 1024)[perm].transpose(1, 0, 2, 3))

    seg_end = np.searchsorted(seg, seg, side="right").astype(np.int64)
    iota = np.broadcast_to(np.arange(512, dtype=np.float32), (P, 512)).copy()
    segrel = np.zeros((P, NT, NJ), dtype=np.float32)
    for i in range(NT):
        for j in range(NJ):
            segrel[:, i, j] = seg_end[P * i:P * i + P] - 512.0 * j

    in_maps = []
    for c in range(NCORES):
        h0, h1 = 2 * c, 2 * c + 1
        g = c // 2
        wqg = np.concatenate([
            wq[:, h0 * 256: h0 * 256 + 128],
            wq[:, h1 * 256: h1 * 256 + 128],
            wq[:, h0 * 256 + 128: h0 * 256 + 256],
            wq[:, h1 * 256 + 128: h1 * 256 + 256],
        ], axis=1)
        wqg_p = np.ascontiguousarray(wqg.reshape(DT, P, 512).transpose(1, 0, 2))
        wkv = np.concatenate([
            wk[:, g * 128:(g + 1) * 128], wv[:, g * 128:(g + 1) * 128]], axis=1)
        wkv_p = np.ascontiguousarray(wkv.reshape(DT, P, 256).transpose(1, 0, 2))
        m = {
            "hT": hT, "wqg": wqg_p, "wkv": wkv_p, "wo": wo_p,
            "tblq": tblq, "iota": iota, "segrel": segrel,
        }
        if not unit_w:
            m["wqk"] = wqk
        in_maps.append(m)
    return in_maps, seg_end, unit_w


def kernel(**inputs) -> np.ndarray:
    in_maps, seg_end, unit_w = _host_prep(**inputs)
    key = (_tile_flags(seg_end), unit_w)
    if key not in _program_cache:
        _program_cache[key] = _build_program(key)
    nc = _program_cache[key]
    res = run_bass_kernel_spmd(nc, in_maps, list(range(NCORES)))
    out = np.concatenate([res.results[c]["out"] for c in range(NCORES)], axis=0)
    return out[None].astype(np.float32)
